# revision 7
# baseline (speedup 1.0000x reference)
"""CWT (GMW filterbank) Trainium2 kernel, v2.

Computes Wx = ifft(Psih * fft(reflect_pad(x)))[..., N1:N1+L] for
x (32, 2048) f32, Psih (256, 4096) f32 -> out (32, 256, 2048) complex64.

Strategy (8 NeuronCores, data-parallel over batch, 4 rows/core):
  - Forward DFT via Cooley-Tukey 4096 = 128 x 32 (DIT): reflect pad is
    materialized by 3 region DMAs + a DVE inner-reversal (the n1-row
    permutation this induces is baked into the stage-1 DFT-128 weights).
    Stage 1 = one [128x128] matmul pair (fp32), twiddle on DVE, one PE
    transpose pair, stage 2 = DFT-32 as a block-diagonal-over-batch
    [128x64] matmul quad (fp32).  ~4 us PE vs ~55 us for the dense DFT,
    and no 16 MB/iteration DFT-matrix HBM traffic.
  - P = Psih (.) xh in fp16 (pre-scaled by 1/4096 via the stage-2
    constants so E can be stored as raw +-1-range phases; avoids fp16
    subnormals).  re on DVE, im on GPSIMD.
  - Banded inverse DFT in fp16 (same PE speed as bf16, 8x the mantissa):
    per-octave k-tile bands at threshold 5e-3 (29 (octave,ktile) pairs).
    Mirror symmetry E[k, 4096-n] = conj(E[k, n]) halves the matmuls:
    U/V/W/Z accumulate over n in [1024, 2048) only; right-half outputs
    assembled with reversed APs.  E tiles persist in SBUF across the
    rep loop (loaded once in the prologue).
  - The 8 concurrent PSUM accumulation groups (U,V,W,Z x 2 n-tiles) map
    1:1 onto the 8 PSUM banks.  start=True clears has_written for the
    whole bank, so interleaved groups MUST be bank-disjoint.
  - Center column n=2048 (self-mirror): per-octave (-1)^p projections of
    P emitted at body end (58 LDW-bound matmuls overlapping the next
    iteration's P-gen via the For_i pipeline), written by 32 tiny
    PSUM->DRAM DMAs.
  - Mirror assembly: V/W copies on ACT (scalar), left combines on DVE,
    right (reversed) combines on GPSIMD; outputs DMA out as
    (b, a, n, 2) f32 in 4x 32-partition chunks to spread HWDGE queues.

Build notes inherited from v1 (hard-won):
  - Use bacc.Bacc() + nc.compile(): plain bass.Bass() fails walrus
    codegen with "Too many sync wait commands".
  - DVE tensor_tensor may read only ONE operand from PSUM.
  - DVE/ACT/GPSIMD are partition-locked; partition permutations need
    DMA or PE transposes (or host-side constant permutation).
  - A single big out-DMA serializes on one HWDGE queue.
"""

import numpy as np

import concourse.bass as bass
import concourse.bacc as bacc
import concourse.mybir as mybir
import concourse.tile as tile
from concourse.bass_utils import run_bass_kernel_spmd

B = 32          # batch
L = 2048        # signal length
UP = 4096       # padded length
N1 = 1024       # left pad (slice offset)
NA = 256        # scales
NV = 32         # voices/octave
NO = 8          # octaves
KF = 2048       # used frequency bins
NC = 8          # cores
BPC = B // NC   # batch rows per core (4)
KT = KF // 128  # k tiles (16)
NTILE = 512     # output columns per matmul
BAND_THRESH = 5e-3

_CACHE = {}


def _host_constants(Psih):
    """FFT stage constants, E filter-bank tiles (fp16), band table."""
    # stage-1 DFT-128 weights with the n1 partition permutation induced by
    # the staged reflect-pad load (regions 1/3 land with n1 reversed)
    n1_of_p = np.arange(128)
    n1_of_p[0:32] = 31 - np.arange(32)
    n1_of_p[96:128] = 223 - np.arange(96, 128)
    th = 2 * np.pi * n1_of_p[:, None] * np.arange(128)[None, :] / 128
    w128_dev = np.stack([np.cos(th), -np.sin(th)], 1).astype(np.float32)

    # twiddle T[k1, n2] = exp(-2i pi k1 n2 / 4096)
    tt = 2 * np.pi * np.arange(128)[:, None] * np.arange(32)[None, :] / UP
    tw_dev = np.stack([np.cos(tt), -np.sin(tt)], 1).astype(np.float32)

    # stage-2 DFT-32 block-diagonal over batch, cols k2 < 16 (k < 2048),
    # scaled by 1/UP so xh (hence P) carries the ifft normalization
    t2 = 2 * np.pi * np.arange(32)[:, None] * np.arange(16)[None, :] / 32
    C32 = np.cos(t2) / UP
    S32 = np.sin(t2) / UP
    c32blk = np.zeros((128, 3, 64), np.float32)
    for b in range(BPC):
        c32blk[32 * b:32 * b + 32, 0, 16 * b:16 * b + 16] = C32
        c32blk[32 * b:32 * b + 32, 1, 16 * b:16 * b + 16] = S32
        c32blk[32 * b:32 * b + 32, 2, 16 * b:16 * b + 16] = -S32

    id128 = np.eye(128, dtype=np.float32)

    # E tiles: raw phases (x UP vs the true inverse-DFT matrix; the 1/UP
    # lives in c32blk): E'[k, n] = exp(2i pi k n / UP), n in [N1, N1+L)
    kk = np.arange(KF)[:, None]
    nn = np.arange(N1, N1 + L // 2)[None, :]
    ph = 2 * np.pi * kk * nn / UP
    Er = np.cos(ph).reshape(KT, 128, 2, NTILE)
    Ei = np.sin(ph).reshape(KT, 128, 2, NTILE)
    e_dev = np.empty((2, KT, 128, 2, NTILE), dtype=np.float16)
    e_dev[:, :, :, 0, :] = Er.transpose(2, 0, 1, 3).astype(np.float16)
    e_dev[:, :, :, 1, :] = Ei.transpose(2, 0, 1, 3).astype(np.float16)

    # center column: E'[k, 2048] = (-1)^k, independent of kt
    epm_dev = ((-1.0) ** (np.arange(128) % 2)).astype(np.float16)[:, None]

    # PsihT device layout: (k_in 128, kt, a 256) f32
    psiht = np.ascontiguousarray(
        Psih[:, :KF].T.reshape(KT, 128, NA).transpose(1, 0, 2)
    ).astype(np.float16)

    bands = []
    for o in range(NO):
        sub = Psih[NV * o:NV * (o + 1), :KF]
        ks = np.nonzero((sub > BAND_THRESH * 2.0).any(axis=0))[0]
        bands.append((int(ks.min()) // 128, int(ks.max()) // 128 + 1))

    return w128_dev, tw_dev, c32blk, id128, e_dev, epm_dev, psiht, bands


def _rev_ap(ap2d, last_col, count):
    """Columns [last_col, last_col-1, ...] of a [128, C] AP."""
    return bass.AP(
        ap2d.tensor,
        ap2d.offset + last_col * ap2d.ap[-1][0],
        [list(ap2d.ap[0]), [-ap2d.ap[-1][0], count]],
    )


def _rev_inner(ap3, n):
    """Reverse the innermost dim (size n) of a 3-dim AP."""
    return bass.AP(
        ap3.tensor, ap3.offset + (n - 1) * ap3.ap[-1][0],
        [list(ap3.ap[0]), list(ap3.ap[1]), [-ap3.ap[-1][0], n]],
    )


def _build_program(w128_dev, tw_dev, c32blk, id128, e_dev, epm_dev, bands,
                   reps=1, variant="full"):
    f32 = mybir.dt.float32
    fp16 = mybir.dt.float16

    nc = bacc.Bacc()
    x_in = nc.dram_tensor("x", [BPC, L], f32, kind="ExternalInput")
    psih_in = nc.dram_tensor("psiht", [128, KT, NA], fp16, kind="ExternalInput")
    out_t = nc.dram_tensor("out", [BPC, NA, L, 2], f32, kind="ExternalOutput")

    w128_c = nc.inline_tensor(w128_dev, name="w128c")
    tw_c = nc.inline_tensor(tw_dev, name="twc")
    c32_c = nc.inline_tensor(c32blk, name="c32c")
    id_c = nc.inline_tensor(id128, name="idc")
    e_c = nc.inline_tensor(e_dev, name="econst")
    epm_c = nc.inline_tensor(epm_dev, name="epmc")

    with tile.TileContext(nc) as tc:
        with (
            tc.tile_pool(name="persist", bufs=1) as persist,
            tc.tile_pool(name="pfix", bufs=1) as pfix,
            tc.tile_pool(name="stg", bufs=3) as stgp,
            tc.tile_pool(name="ps_m", bufs=2, space="PSUM") as ps_m,
        ):
            # ---- prologue: inputs + persistent constants ----
            xpt = persist.tile([128, BPC, 32], f32, tag="xp")
            s13 = persist.tile([128, BPC, 32], f32, tag="s13")
            for b in range(BPC):
                nc.sync.dma_start(
                    out=xpt[32:96, b],
                    in_=bass.AP(x_in, L * b, [[32, 64], [1, 32]]),
                )
                nc.sync.dma_start(
                    out=s13[0:32, b],
                    in_=bass.AP(x_in, L * b + 1, [[32, 32], [1, 32]]),
                )
                nc.sync.dma_start(
                    out=s13[96:128, b],
                    in_=bass.AP(x_in, L * b + 1023, [[32, 32], [1, 32]]),
                )
            nc.vector.tensor_copy(out=xpt[0:32], in_=_rev_inner(s13[0:32], 32))
            nc.vector.tensor_copy(out=xpt[96:128], in_=_rev_inner(s13[96:128], 32))

            psih_sb = persist.tile([128, KT, NA], fp16, tag="psih")
            nc.sync.dma_start(out=psih_sb, in_=psih_in[:])
            # funnel: absorb the psih DMA wait into DVE's clock
            scratch = persist.tile([1, 4], f32, tag="scratch")
            nc.vector.tensor_copy(out=scratch[0:1, 0:1], in_=psih_sb[0:1, 0, 0:1])

            w128_sb = persist.tile([128, 2, 128], f32, tag="w128")
            nc.sync.dma_start(out=w128_sb, in_=w128_c[:])
            tw_sb = persist.tile([128, 2, 32], f32, tag="tw")
            nc.sync.dma_start(out=tw_sb, in_=tw_c[:])
            c32_sb = persist.tile([128, 3, 64], f32, tag="c32")
            nc.sync.dma_start(out=c32_sb, in_=c32_c[:])
            id_sb = persist.tile([128, 128], f32, tag="id")
            nc.sync.dma_start(out=id_sb, in_=id_c[:])
            epm_sb = persist.tile([128, 1], fp16, tag="epm")
            nc.sync.dma_start(out=epm_sb, in_=epm_c[:])

            etiles = {}
            for lnt in range(2):
                for kt in range(KT):
                    et = persist.tile([128, 2, NTILE], fp16, tag=f"e{lnt}_{kt}")
                    nc.sync.dma_start(out=et, in_=e_c[lnt, kt])
                    etiles[(lnt, kt)] = et

            # dummy transpose absorbs the identity-DMA wait on PE
            dmy = ps_m.tile([128, 2, NTILE], f32, tag="uv", name="dmy")
            nc.tensor.transpose(dmy[:, 0, 0:128], id_sb, id_sb)

            def body():
                _emit_body(
                    nc, bands, out_t, persist, pfix, stgp, ps_m,
                    xpt, psih_sb, w128_sb, tw_sb, c32_sb, id_sb, epm_sb,
                    etiles, f32, fp16, variant,
                )

            if reps == 1:
                body()
            else:
                with tc.For_i(0, reps, 1):
                    body()
    nc.compile()
    return nc


def _emit_body(nc, bands, out_t, persist, pfix, stgp, ps_m,
               xpt, psih_sb, w128_sb, tw_sb, c32_sb, id_sb, epm_sb,
               etiles, f32, fp16, variant):
    skip_fwd = "nofwd" in variant
    skip_out = "noout" in variant
    skip_mm = "nomm" in variant
    skip_ctr = "noctr" in variant

    # ---- forward FFT (one ps_m "uv" slot hosts all fwd psum) ----
    xh_all = persist.tile([128, 2, BPC, KT], fp16, tag="xh")
    if not skip_fwd:
        fwd = ps_m.tile([128, 2, NTILE], f32, tag="uv", name="fwd")
        bre = fwd[:, 0, 0:128]
        bim = fwd[:, 1, 0:128]
        xp_flat = xpt.rearrange("p b n -> p (b n)")
        nc.tensor.matmul(bre, w128_sb[:, 0, :], xp_flat, start=True, stop=True)
        nc.tensor.matmul(bim, w128_sb[:, 1, :], xp_flat, start=True, stop=True)

        dd = pfix.tile([128, 2, BPC, 32], f32, tag="dd")
        t1 = pfix.tile([128, BPC, 32], f32, tag="t1")
        t2 = pfix.tile([128, BPC, 32], f32, tag="t2")
        brr = bre.rearrange("p (b n) -> p b n", b=BPC)
        bir = bim.rearrange("p (b n) -> p b n", b=BPC)
        tre = tw_sb[:, 0, None, :].to_broadcast((128, BPC, 32))
        tim = tw_sb[:, 1, None, :].to_broadcast((128, BPC, 32))
        nc.vector.tensor_tensor(t1, brr, tre, mybir.AluOpType.mult)
        nc.vector.tensor_tensor(t2, bir, tim, mybir.AluOpType.mult)
        nc.vector.tensor_sub(dd[:, 0], t1, t2)
        nc.vector.tensor_tensor(t1, brr, tim, mybir.AluOpType.mult)
        nc.vector.tensor_tensor(t2, bir, tre, mybir.AluOpType.mult)
        nc.vector.tensor_add(dd[:, 1], t1, t2)

        dtp_re = fwd[:, 0, 128:256]
        dtp_im = fwd[:, 1, 128:256]
        nc.tensor.transpose(dtp_re, dd[:, 0].rearrange("p b n -> p (b n)"), id_sb)
        nc.tensor.transpose(dtp_im, dd[:, 1].rearrange("p b n -> p (b n)"), id_sb)
        dts = pfix.tile([128, 2, 128], f32, tag="dts")
        nc.scalar.copy(out=dts[:, 0, :], in_=dtp_re)
        nc.scalar.copy(out=dts[:, 1, :], in_=dtp_im)

        xre = fwd[:, 0, 256:320]
        xim = fwd[:, 1, 256:320]
        nc.tensor.matmul(xre, dts[:, 0, :], c32_sb[:, 0, :], start=True, stop=False)
        nc.tensor.matmul(xim, dts[:, 0, :], c32_sb[:, 2, :], start=True, stop=False)
        nc.tensor.matmul(xre, dts[:, 1, :], c32_sb[:, 1, :], start=False, stop=True)
        nc.tensor.matmul(xim, dts[:, 1, :], c32_sb[:, 0, :], start=False, stop=True)
        nc.vector.tensor_copy(
            out=xh_all.rearrange("p c b k -> p c (b k)"),
            in_=fwd[:, :, 256:320],
        )

    # ---- P generation, high kt first (octave 0's band is ready first).
    # All on DVE in fp16 (GPSIMD has multi-us per-op overhead).  Band sums
    # for the center column accumulate incrementally so the ctr matmuls at
    # body end never touch the P tiles (keeps the For_i pipeline free).
    P_re, P_im = {}, {}
    psum_o = {} if not skip_mm else None
    for kt in reversed(range(KT)):
        psih_ap = (
            psih_sb[:, kt, :]
            .rearrange("p (o a) -> p o a", o=NO)[:, :, None, :]
            .to_broadcast((128, NO, BPC, NV))
        )
        for comp, dst in ((0, P_re), (1, P_im)):
            pt = pfix.tile([128, NO * BPC * NV], fp16, tag=f"P{comp}_{kt}")
            if skip_fwd:
                nc.vector.tensor_copy(
                    out=pt.rearrange("p (o b a) -> p o b a", o=NO, b=BPC),
                    in_=psih_ap,
                )
            else:
                xh_ap = (
                    xh_all[:, comp, :, kt][:, None, :, None]
                    .to_broadcast((128, NO, BPC, NV))
                )
                nc.vector.tensor_tensor(
                    pt.rearrange("p (o b a) -> p o b a", o=NO, b=BPC),
                    psih_ap,
                    xh_ap,
                    mybir.AluOpType.mult,
                )
            dst[kt] = pt
        if psum_o is not None:
            for o in range(NO):
                klo, khi = bands[o]
                if not (klo <= kt < khi):
                    continue
                osl = slice(o * 128, (o + 1) * 128)
                if o not in psum_o:
                    psum_o[o] = pfix.tile([128, 2, 128], fp16,
                                          tag=f"psum{o}", bufs=2,
                                          name=f"psum{o}")
                ps_t = psum_o[o]
                for comp, src_t in ((0, P_re[kt]), (1, P_im[kt])):
                    if kt == khi - 1:
                        nc.vector.tensor_copy(out=ps_t[:, comp, :],
                                              in_=src_t[:, osl])
                    else:
                        nc.vector.tensor_add(ps_t[:, comp, :],
                                             ps_t[:, comp, :], src_t[:, osl])

    # ---- banded mirror inverse DFT ----
    for o in range(NO):
        if skip_mm:
            continue
        klo, khi = bands[o]
        kts = list(range(klo, khi))
        osl = slice(o * 128, (o + 1) * 128)
        uv0 = ps_m.tile([128, 2, NTILE], f32, tag="uv", name="uv0")
        wz0 = ps_m.tile([128, 2, NTILE], f32, tag="wz", name="wz0")
        uv1 = ps_m.tile([128, 2, NTILE], f32, tag="uv", name="uv1")
        wz1 = ps_m.tile([128, 2, NTILE], f32, tag="wz", name="wz1")
        for j, kt in enumerate(kts):
            st, sp = (j == 0), (j == len(kts) - 1)
            pr = P_re[kt][:, osl]
            pi = P_im[kt][:, osl]
            er0 = etiles[(0, kt)][:, 0, :]
            ei0 = etiles[(0, kt)][:, 1, :]
            er1 = etiles[(1, kt)][:, 0, :]
            ei1 = etiles[(1, kt)][:, 1, :]
            # stationary-reuse ordering: 4 streams per LDW
            nc.tensor.matmul(uv0[:, 0, :], pr, er0, start=st, stop=sp)
            nc.tensor.matmul(uv1[:, 0, :], pr, er1, start=st, stop=sp)
            nc.tensor.matmul(wz0[:, 0, :], pr, ei0, start=st, stop=sp)
            nc.tensor.matmul(wz1[:, 0, :], pr, ei1, start=st, stop=sp)
            nc.tensor.matmul(uv0[:, 1, :], pi, ei0, start=st, stop=sp)
            nc.tensor.matmul(uv1[:, 1, :], pi, ei1, start=st, stop=sp)
            nc.tensor.matmul(wz0[:, 1, :], pi, er0, start=st, stop=sp)
            nc.tensor.matmul(wz1[:, 1, :], pi, er1, start=st, stop=sp)

        # U=uv[:,0], V=uv[:,1], W=wz[:,0], Z=wz[:,1]
        vw0 = stgp.tile([128, 2, NTILE], f32, tag="vw")
        nc.scalar.copy(out=vw0[:, 0, :], in_=uv0[:, 1, :])
        nc.scalar.copy(out=vw0[:, 1, :], in_=wz0[:, 0, :])
        stgL0 = stgp.tile([128, NTILE, 2], f32, tag="stg")
        nc.vector.tensor_sub(stgL0[:, :, 0], uv0[:, 0, :], vw0[:, 0, :])
        nc.vector.tensor_add(stgL0[:, :, 1], wz0[:, 1, :], vw0[:, 1, :])
        _dma_out(nc, out_t, stgL0, o, 0, skip_out)

        vw1 = stgp.tile([128, 2, NTILE], f32, tag="vw")
        nc.scalar.copy(out=vw1[:, 0, :], in_=uv1[:, 1, :])
        nc.scalar.copy(out=vw1[:, 1, :], in_=wz1[:, 0, :])
        stgL1 = stgp.tile([128, NTILE, 2], f32, tag="stg")
        nc.vector.tensor_sub(stgL1[:, :, 0], uv1[:, 0, :], vw1[:, 0, :])
        nc.vector.tensor_add(stgL1[:, :, 1], wz1[:, 1, :], vw1[:, 1, :])
        _dma_out(nc, out_t, stgL1, o, 1, skip_out)

        # right tile 1: n in [2560, 3072) mirrors left tile 0
        stgR1 = stgp.tile([128, NTILE, 2], f32, tag="stg")
        nc.vector.tensor_add(stgR1[:, 1:NTILE, 0],
                             _rev_ap(uv0[:, 0, :], NTILE - 1, NTILE - 1),
                             _rev_ap(vw0[:, 0, :], NTILE - 1, NTILE - 1))
        nc.vector.tensor_sub(stgR1[:, 1:NTILE, 1],
                             _rev_ap(wz0[:, 1, :], NTILE - 1, NTILE - 1),
                             _rev_ap(vw0[:, 1, :], NTILE - 1, NTILE - 1))
        nc.vector.tensor_add(stgR1[:, 0:1, 0], uv1[:, 0, 0:1], vw1[:, 0, 0:1])
        nc.vector.tensor_sub(stgR1[:, 0:1, 1], wz1[:, 1, 0:1], vw1[:, 1, 0:1])
        _dma_out(nc, out_t, stgR1, o, 3, skip_out)

        # right tile 0: n in (2048, 2560) mirrors left tile 1; col 0 (the
        # self-mirrored n=2048 column) is written separately at body end
        stgR0 = stgp.tile([128, NTILE, 2], f32, tag="stg")
        nc.vector.tensor_add(stgR0[:, 1:NTILE, 0],
                             _rev_ap(uv1[:, 0, :], NTILE - 1, NTILE - 1),
                             _rev_ap(vw1[:, 0, :], NTILE - 1, NTILE - 1))
        nc.vector.tensor_sub(stgR0[:, 1:NTILE, 1],
                             _rev_ap(wz1[:, 1, :], NTILE - 1, NTILE - 1),
                             _rev_ap(vw1[:, 1, :], NTILE - 1, NTILE - 1))
        if not skip_out:
            for bl in range(BPC):
                nc.sync.dma_start(
                    out=out_t[bl, NV * o:NV * (o + 1),
                              2 * NTILE + 1:3 * NTILE, :],
                    in_=stgR0[NV * bl:NV * (bl + 1), 1:NTILE, :],
                )

    # ---- center column n=2048 (out col 1024): ctr = sum_k P * (-1)^p ----
    if not (skip_ctr or skip_mm):
        ctr = ps_m.tile([128, 2, NTILE], f32, tag="uv", name="ctr")
        for o in range(NO):
            nc.tensor.matmul(ctr[:, 0, o:o + 1], psum_o[o][:, 0, :],
                             epm_sb, start=True, stop=True)
            nc.tensor.matmul(ctr[:, 1, o:o + 1], psum_o[o][:, 1, :],
                             epm_sb, start=True, stop=True)
        ctr_sb = pfix.tile([128, 2, NO], f32, tag="ctrsb")
        nc.vector.tensor_copy(out=ctr_sb, in_=ctr[:, :, 0:NO])
        if not skip_out:
            for o in range(NO):
                for bl in range(BPC):
                    nc.sync.dma_start(
                        out=out_t[bl, NV * o:NV * (o + 1), 2 * NTILE, :],
                        in_=bass.AP(
                            ctr_sb.tensor,
                            ctr_sb.offset + NV * bl * ctr_sb.ap[0][0] + o,
                            [[ctr_sb.ap[0][0], NV], [NO, 2]],
                        ),
                    )


def _dma_out(nc, out_t, stg, o, nt, skip_out):
    if skip_out:
        return
    for bl in range(BPC):
        nc.sync.dma_start(
            out=out_t[bl, NV * o:NV * (o + 1), NTILE * nt:NTILE * (nt + 1), :],
            in_=stg[NV * bl:NV * (bl + 1), :, :],
        )


def _get_program(Psih, reps=1, variant="full"):
    key = f"prog{reps}_{variant}"
    if key not in _CACHE:
        if "consts" not in _CACHE:
            _CACHE["consts"] = _host_constants(np.asarray(Psih))
        w128_dev, tw_dev, c32blk, id128, e_dev, epm_dev, psiht, bands = _CACHE["consts"]
        nc = _build_program(w128_dev, tw_dev, c32blk, id128, e_dev, epm_dev,
                            bands, reps=reps, variant=variant)
        _CACHE[key] = (nc, psiht)
    return _CACHE[key]


def kernel(x, Psih=None, **_unused):
    x = np.ascontiguousarray(np.asarray(x), dtype=np.float32)
    if Psih is None:
        raise ValueError("Psih input required")
    nc, psiht = _get_program(Psih)
    in_maps = [
        {"x": np.ascontiguousarray(x[BPC * c:BPC * (c + 1)]), "psiht": psiht}
        for c in range(NC)
    ]
    res = run_bass_kernel_spmd(nc, in_maps, core_ids=list(range(NC)))
    out = np.concatenate([r["out"] for r in res.results], axis=0)
    return out.view(np.complex64)[..., 0]


def bench(x, Psih, iters=20, reps=1, variant="full"):
    """Run the kernel repeatedly on-device; returns (out_complex, times_ns)."""
    import time
    import jax
    from jax.sharding import Mesh, PartitionSpec
    from jax.experimental.shard_map import shard_map
    from concourse import bass2jax

    x = np.ascontiguousarray(np.asarray(x), dtype=np.float32)
    nc, psiht = _get_program(Psih, reps=reps, variant=variant)
    bass2jax.install_neuronx_cc_hook()

    part_name = nc.partition_id_tensor.name if nc.partition_id_tensor else None
    in_names, out_names, out_avals = [], [], []
    for alloc in nc.m.functions[0].allocations:
        if not isinstance(alloc, mybir.MemoryLocationSet):
            continue
        name = alloc.memorylocations[0].name
        if alloc.kind == "ExternalInput":
            if name != part_name:
                in_names.append(name)
        elif alloc.kind == "ExternalOutput":
            out_names.append(name)
            out_avals.append(
                jax.core.ShapedArray(
                    tuple(alloc.tensor_shape), mybir.dt.np(alloc.dtype)
                )
            )
    n_params = len(in_names)
    all_names = in_names + out_names
    if part_name is not None:
        all_names = all_names + [part_name]

    def _body(*args):
        operands = list(args)
        if part_name is not None:
            operands.append(bass2jax.partition_id_tensor())
        outs = bass2jax._bass_exec_p.bind(
            *operands,
            out_avals=tuple(out_avals),
            in_names=tuple(all_names),
            out_names=tuple(out_names),
            lowering_input_output_aliases=(),
            sim_require_finite=True,
            sim_require_nnan=True,
            nc=nc,
        )
        return tuple(outs)

    devices = jax.devices()[:NC]
    mesh = Mesh(np.asarray(devices), ("core",))
    nin = n_params + len(out_names)
    fn = jax.jit(
        shard_map(
            _body,
            mesh=mesh,
            in_specs=(PartitionSpec("core"),) * nin,
            out_specs=(PartitionSpec("core"),) * len(out_names),
            check_rep=False,
        ),
        keep_unused=True,
    )
    in_map = {"x": x, "psiht": np.concatenate([psiht] * NC, axis=0)}
    concat_in = [in_map[n] for n in in_names]
    concat_zeros = [
        np.zeros((NC * a.shape[0], *a.shape[1:]), a.dtype) for a in out_avals
    ]
    sharding = jax.sharding.NamedSharding(mesh, PartitionSpec("core"))
    args = [jax.device_put(a, sharding) for a in concat_in + concat_zeros]
    out_arrs = jax.block_until_ready(fn(*args))  # compile + first run
    times = []
    for _ in range(iters):
        t0 = time.perf_counter()
        out_arrs = jax.block_until_ready(fn(*args))
        times.append((time.perf_counter() - t0) * 1e9)
    out = np.asarray(out_arrs[0]).reshape(NC, BPC, NA, L, 2).reshape(B, NA, L, 2)
    return out.view(np.complex64)[..., 0], times


# revision 9
# speedup vs baseline: 1.3615x; 1.3615x over previous
"""CWT (GMW filterbank) Trainium2 kernel, v2.

Computes Wx = ifft(Psih * fft(reflect_pad(x)))[..., N1:N1+L] for
x (32, 2048) f32, Psih (256, 4096) f32 -> out (32, 256, 2048) complex64.

Strategy (8 NeuronCores, data-parallel over batch, 4 rows/core):
  - Forward DFT via Cooley-Tukey 4096 = 128 x 32 (DIT): reflect pad is
    materialized by 3 region DMAs + a DVE inner-reversal (the n1-row
    permutation this induces is baked into the stage-1 DFT-128 weights).
    Stage 1 = one [128x128] matmul pair (fp32), twiddle on DVE, one PE
    transpose pair, stage 2 = DFT-32 as a block-diagonal-over-batch
    [128x64] matmul quad (fp32).  ~4 us PE vs ~55 us for the dense DFT,
    and no 16 MB/iteration DFT-matrix HBM traffic.
  - P = Psih (.) xh in fp16 (pre-scaled by 1/4096 via the stage-2
    constants so E can be stored as raw +-1-range phases; avoids fp16
    subnormals).  re on DVE, im on GPSIMD.
  - Banded inverse DFT in fp16 (same PE speed as bf16, 8x the mantissa):
    per-octave k-tile bands at threshold 5e-3 (29 (octave,ktile) pairs).
    Mirror symmetry E[k, 4096-n] = conj(E[k, n]) halves the matmuls:
    U/V/W/Z accumulate over n in [1024, 2048) only; right-half outputs
    assembled with reversed APs.  E tiles persist in SBUF across the
    rep loop (loaded once in the prologue).
  - The 8 concurrent PSUM accumulation groups (U,V,W,Z x 2 n-tiles) map
    1:1 onto the 8 PSUM banks.  start=True clears has_written for the
    whole bank, so interleaved groups MUST be bank-disjoint.
  - Center column n=2048 (self-mirror): per-octave (-1)^p projections of
    P emitted at body end (58 LDW-bound matmuls overlapping the next
    iteration's P-gen via the For_i pipeline), written by 32 tiny
    PSUM->DRAM DMAs.
  - Mirror assembly: V/W copies on ACT (scalar), left combines on DVE,
    right (reversed) combines on GPSIMD; outputs DMA out as
    (b, a, n, 2) f32 in 4x 32-partition chunks to spread HWDGE queues.

Build notes inherited from v1 (hard-won):
  - Use bacc.Bacc() + nc.compile(): plain bass.Bass() fails walrus
    codegen with "Too many sync wait commands".
  - DVE tensor_tensor may read only ONE operand from PSUM.
  - DVE/ACT/GPSIMD are partition-locked; partition permutations need
    DMA or PE transposes (or host-side constant permutation).
  - A single big out-DMA serializes on one HWDGE queue.
"""

import numpy as np

import concourse.bass as bass
import concourse.bacc as bacc
import concourse.mybir as mybir
import concourse.tile as tile
from concourse.bass_utils import run_bass_kernel_spmd

B = 32          # batch
L = 2048        # signal length
UP = 4096       # padded length
N1 = 1024       # left pad (slice offset)
NA = 256        # scales
NV = 32         # voices/octave
NO = 8          # octaves
KF = 2048       # used frequency bins
NC = 8          # cores
BPC = B // NC   # batch rows per core (4)
KT = KF // 128  # k tiles (16)
NTILE = 512     # output columns per matmul
BAND_THRESH = 5e-3

_CACHE = {}


def _host_constants(Psih):
    """FFT stage constants, E filter-bank tiles (fp16), band table."""
    # stage-1 DFT-128 weights with the n1 partition permutation induced by
    # the staged reflect-pad load (regions 1/3 land with n1 reversed)
    n1_of_p = np.arange(128)
    n1_of_p[0:32] = 31 - np.arange(32)
    n1_of_p[96:128] = 223 - np.arange(96, 128)
    th = 2 * np.pi * n1_of_p[:, None] * np.arange(128)[None, :] / 128
    w128_dev = np.stack([np.cos(th), -np.sin(th)], 1).astype(np.float32)

    # twiddle T[k1, n2] = exp(-2i pi k1 n2 / 4096)
    tt = 2 * np.pi * np.arange(128)[:, None] * np.arange(32)[None, :] / UP
    tw_dev = np.stack([np.cos(tt), -np.sin(tt)], 1).astype(np.float32)

    # stage-2 DFT-32 block-diagonal over batch, cols k2 < 16 (k < 2048),
    # scaled by 1/UP so xh (hence P) carries the ifft normalization
    t2 = 2 * np.pi * np.arange(32)[:, None] * np.arange(16)[None, :] / 32
    C32 = np.cos(t2) / UP
    S32 = np.sin(t2) / UP
    c32blk = np.zeros((128, 3, 64), np.float32)
    for b in range(BPC):
        c32blk[32 * b:32 * b + 32, 0, 16 * b:16 * b + 16] = C32
        c32blk[32 * b:32 * b + 32, 1, 16 * b:16 * b + 16] = S32
        c32blk[32 * b:32 * b + 32, 2, 16 * b:16 * b + 16] = -S32

    id128 = np.eye(128, dtype=np.float32)

    # E tiles: raw phases (x UP vs the true inverse-DFT matrix; the 1/UP
    # lives in c32blk): E'[k, n] = exp(2i pi k n / UP), n in [N1, N1+L)
    kk = np.arange(KF)[:, None]
    nn = np.arange(N1, N1 + L // 2)[None, :]
    ph = 2 * np.pi * kk * nn / UP
    Er = np.cos(ph).reshape(KT, 128, 2, NTILE)
    Ei = np.sin(ph).reshape(KT, 128, 2, NTILE)
    e_dev = np.empty((2, KT, 128, 2, NTILE), dtype=np.float16)
    e_dev[:, :, :, 0, :] = Er.transpose(2, 0, 1, 3).astype(np.float16)
    e_dev[:, :, :, 1, :] = Ei.transpose(2, 0, 1, 3).astype(np.float16)

    # center column: E'[k, 2048] = (-1)^k, independent of kt
    epm_dev = ((-1.0) ** (np.arange(128) % 2)).astype(np.float16)[:, None]

    # PsihT device layout: (k_in 128, kt, a 256) f32
    psiht = np.ascontiguousarray(
        Psih[:, :KF].T.reshape(KT, 128, NA).transpose(1, 0, 2)
    ).astype(np.float16)

    bands = []
    for o in range(NO):
        sub = Psih[NV * o:NV * (o + 1), :KF]
        ks = np.nonzero((sub > BAND_THRESH * 2.0).any(axis=0))[0]
        bands.append((int(ks.min()) // 128, int(ks.max()) // 128 + 1))

    return w128_dev, tw_dev, c32blk, id128, e_dev, epm_dev, psiht, bands


def _rev_ap(ap2d, last_col, count):
    """Columns [last_col, last_col-1, ...] of a [128, C] AP."""
    return bass.AP(
        ap2d.tensor,
        ap2d.offset + last_col * ap2d.ap[-1][0],
        [list(ap2d.ap[0]), [-ap2d.ap[-1][0], count]],
    )


def _rev_inner(ap3, n):
    """Reverse the innermost dim (size n) of a 3-dim AP."""
    return bass.AP(
        ap3.tensor, ap3.offset + (n - 1) * ap3.ap[-1][0],
        [list(ap3.ap[0]), list(ap3.ap[1]), [-ap3.ap[-1][0], n]],
    )


def _build_program(w128_dev, tw_dev, c32blk, id128, e_dev, epm_dev, bands,
                   reps=1, variant="full"):
    f32 = mybir.dt.float32
    fp16 = mybir.dt.float16

    nc = bacc.Bacc()
    x_in = nc.dram_tensor("x", [BPC, L], f32, kind="ExternalInput")
    psih_in = nc.dram_tensor("psiht", [128, KT, NA], fp16, kind="ExternalInput")
    out_t = nc.dram_tensor("out", [BPC, NA, L, 2], f32, kind="ExternalOutput")

    w128_c = nc.inline_tensor(w128_dev, name="w128c")
    tw_c = nc.inline_tensor(tw_dev, name="twc")
    c32_c = nc.inline_tensor(c32blk, name="c32c")
    id_c = nc.inline_tensor(id128, name="idc")
    e_c = nc.inline_tensor(e_dev, name="econst")
    epm_c = nc.inline_tensor(epm_dev, name="epmc")

    with tile.TileContext(nc) as tc:
        with (
            tc.tile_pool(name="persist", bufs=1) as persist,
            tc.tile_pool(name="pfix", bufs=1) as pfix,
            tc.tile_pool(name="stg", bufs=3) as stgp,
            tc.tile_pool(name="ps_m", bufs=2, space="PSUM") as ps_m,
        ):
            # ---- prologue: inputs + persistent constants ----
            xpt = persist.tile([128, BPC, 32], f32, tag="xp")
            s13 = persist.tile([128, BPC, 32], f32, tag="s13")
            for b in range(BPC):
                nc.sync.dma_start(
                    out=xpt[32:96, b],
                    in_=bass.AP(x_in, L * b, [[32, 64], [1, 32]]),
                )
                nc.sync.dma_start(
                    out=s13[0:32, b],
                    in_=bass.AP(x_in, L * b + 1, [[32, 32], [1, 32]]),
                )
                nc.sync.dma_start(
                    out=s13[96:128, b],
                    in_=bass.AP(x_in, L * b + 1023, [[32, 32], [1, 32]]),
                )
            nc.vector.tensor_copy(out=xpt[0:32], in_=_rev_inner(s13[0:32], 32))
            nc.vector.tensor_copy(out=xpt[96:128], in_=_rev_inner(s13[96:128], 32))

            psih_sb = persist.tile([128, KT, NA], fp16, tag="psih")
            nc.sync.dma_start(out=psih_sb, in_=psih_in[:])
            # funnel: absorb the psih DMA wait into DVE's clock
            scratch = persist.tile([1, 4], f32, tag="scratch")
            nc.vector.tensor_copy(out=scratch[0:1, 0:1], in_=psih_sb[0:1, 0, 0:1])

            w128_sb = persist.tile([128, 2, 128], f32, tag="w128")
            nc.sync.dma_start(out=w128_sb, in_=w128_c[:])
            tw_sb = persist.tile([128, 2, 32], f32, tag="tw")
            nc.sync.dma_start(out=tw_sb, in_=tw_c[:])
            c32_sb = persist.tile([128, 3, 64], f32, tag="c32")
            nc.sync.dma_start(out=c32_sb, in_=c32_c[:])
            id_sb = persist.tile([128, 128], f32, tag="id")
            nc.sync.dma_start(out=id_sb, in_=id_c[:])
            epm_sb = persist.tile([128, 1], fp16, tag="epm")
            nc.sync.dma_start(out=epm_sb, in_=epm_c[:])

            etiles = {}
            for lnt in range(2):
                for kt in range(KT):
                    et = persist.tile([128, 2, NTILE], fp16, tag=f"e{lnt}_{kt}")
                    nc.sync.dma_start(out=et, in_=e_c[lnt, kt])
                    etiles[(lnt, kt)] = et

            # dummy transpose absorbs the identity-DMA wait on PE
            dmy = ps_m.tile([128, 2, NTILE], f32, tag="uv", name="dmy")
            nc.tensor.transpose(dmy[:, 0, 0:128], id_sb, id_sb)

            def body():
                _emit_body(
                    nc, bands, out_t, persist, pfix, stgp, ps_m,
                    xpt, psih_sb, w128_sb, tw_sb, c32_sb, id_sb, epm_sb,
                    etiles, f32, fp16, variant,
                )

            if reps == 1:
                body()
            else:
                with tc.For_i(0, reps, 1):
                    body()
    nc.compile()
    return nc


def _emit_body(nc, bands, out_t, persist, pfix, stgp, ps_m,
               xpt, psih_sb, w128_sb, tw_sb, c32_sb, id_sb, epm_sb,
               etiles, f32, fp16, variant):
    skip_fwd = "nofwd" in variant
    skip_out = "noout" in variant
    skip_mm = "nomm" in variant
    skip_ctr = "noctr" in variant

    # ---- forward FFT (one ps_m "uv" slot hosts all fwd psum) ----
    xh_all = persist.tile([128, 2, BPC, KT], fp16, tag="xh")
    if not skip_fwd:
        fwd = ps_m.tile([128, 2, NTILE], f32, tag="uv", name="fwd")
        bre = fwd[:, 0, 0:128]
        bim = fwd[:, 1, 0:128]
        xp_flat = xpt.rearrange("p b n -> p (b n)")
        nc.tensor.matmul(bre, w128_sb[:, 0, :], xp_flat, start=True, stop=True)
        nc.tensor.matmul(bim, w128_sb[:, 1, :], xp_flat, start=True, stop=True)

        dd = pfix.tile([128, 2, BPC, 32], f32, tag="dd")
        t1 = pfix.tile([128, BPC, 32], f32, tag="t1")
        t2 = pfix.tile([128, BPC, 32], f32, tag="t2")
        brr = bre.rearrange("p (b n) -> p b n", b=BPC)
        bir = bim.rearrange("p (b n) -> p b n", b=BPC)
        tre = tw_sb[:, 0, None, :].to_broadcast((128, BPC, 32))
        tim = tw_sb[:, 1, None, :].to_broadcast((128, BPC, 32))
        nc.vector.tensor_tensor(t1, brr, tre, mybir.AluOpType.mult)
        nc.vector.tensor_tensor(t2, bir, tim, mybir.AluOpType.mult)
        nc.vector.tensor_sub(dd[:, 0], t1, t2)
        nc.vector.tensor_tensor(t1, brr, tim, mybir.AluOpType.mult)
        nc.vector.tensor_tensor(t2, bir, tre, mybir.AluOpType.mult)
        nc.vector.tensor_add(dd[:, 1], t1, t2)

        dtp_re = fwd[:, 0, 128:256]
        dtp_im = fwd[:, 1, 128:256]
        nc.tensor.transpose(dtp_re, dd[:, 0].rearrange("p b n -> p (b n)"), id_sb)
        nc.tensor.transpose(dtp_im, dd[:, 1].rearrange("p b n -> p (b n)"), id_sb)
        dts = pfix.tile([128, 2, 128], f32, tag="dts")
        nc.scalar.copy(out=dts[:, 0, :], in_=dtp_re)
        nc.scalar.copy(out=dts[:, 1, :], in_=dtp_im)

        xre = fwd[:, 0, 256:320]
        xim = fwd[:, 1, 256:320]
        nc.tensor.matmul(xre, dts[:, 0, :], c32_sb[:, 0, :], start=True, stop=False)
        nc.tensor.matmul(xim, dts[:, 0, :], c32_sb[:, 2, :], start=True, stop=False)
        nc.tensor.matmul(xre, dts[:, 1, :], c32_sb[:, 1, :], start=False, stop=True)
        nc.tensor.matmul(xim, dts[:, 1, :], c32_sb[:, 0, :], start=False, stop=True)
        nc.vector.tensor_copy(
            out=xh_all.rearrange("p c b k -> p c (b k)"),
            in_=fwd[:, :, 256:320],
        )

    # ---- P generation, high kt first (octave 0's band is ready first).
    # All on DVE in fp16 (GPSIMD has multi-us per-op overhead).  Band sums
    # for the center column accumulate incrementally so the ctr matmuls at
    # body end never touch the P tiles (keeps the For_i pipeline free).
    P_re, P_im = {}, {}
    psum_o = {} if not skip_mm else None
    for kt in reversed(range(KT)):
        octs = [o for o in range(NO) if bands[o][0] <= kt < bands[o][1]]
        olo, ohi = min(octs), max(octs) + 1
        nocts = ohi - olo
        psih_ap = (
            psih_sb[:, kt, :]
            .rearrange("p (o a) -> p o a", o=NO)[:, olo:ohi, None, :]
            .to_broadcast((128, nocts, BPC, NV))
        )
        for comp, dst in ((0, P_re), (1, P_im)):
            pt = pfix.tile([128, NO * BPC * NV], fp16, tag=f"P{comp}_{kt}")
            out_ap = pt.rearrange("p (o b a) -> p o b a", o=NO, b=BPC)[:, olo:ohi]
            if skip_fwd:
                nc.vector.tensor_copy(out=out_ap, in_=psih_ap)
            else:
                xh_ap = (
                    xh_all[:, comp, :, kt][:, None, :, None]
                    .to_broadcast((128, nocts, BPC, NV))
                )
                nc.vector.tensor_tensor(
                    out_ap, psih_ap, xh_ap, mybir.AluOpType.mult,
                )
            dst[kt] = pt
        if psum_o is not None:
            for o in range(NO):
                klo, khi = bands[o]
                if not (klo <= kt < khi):
                    continue
                osl = slice(o * 128, (o + 1) * 128)
                if o not in psum_o:
                    psum_o[o] = pfix.tile([128, 2, 128], fp16,
                                          tag=f"psum{o}", bufs=2,
                                          name=f"psum{o}")
                ps_t = psum_o[o]
                for comp, src_t in ((0, P_re[kt]), (1, P_im[kt])):
                    if kt == khi - 1:
                        nc.vector.tensor_copy(out=ps_t[:, comp, :],
                                              in_=src_t[:, osl])
                    else:
                        nc.vector.tensor_add(ps_t[:, comp, :],
                                             ps_t[:, comp, :], src_t[:, osl])

    # ---- banded mirror inverse DFT ----
    for o in range(NO):
        if skip_mm:
            continue
        klo, khi = bands[o]
        kts = list(range(klo, khi))
        osl = slice(o * 128, (o + 1) * 128)
        uv0 = ps_m.tile([128, 2, NTILE], f32, tag="uv", name="uv0")
        wz0 = ps_m.tile([128, 2, NTILE], f32, tag="wz", name="wz0")
        uv1 = ps_m.tile([128, 2, NTILE], f32, tag="uv", name="uv1")
        wz1 = ps_m.tile([128, 2, NTILE], f32, tag="wz", name="wz1")
        for j, kt in enumerate(kts):
            st, sp = (j == 0), (j == len(kts) - 1)
            pr = P_re[kt][:, osl]
            pi = P_im[kt][:, osl]
            er0 = etiles[(0, kt)][:, 0, :]
            ei0 = etiles[(0, kt)][:, 1, :]
            er1 = etiles[(1, kt)][:, 0, :]
            ei1 = etiles[(1, kt)][:, 1, :]
            # stationary-reuse ordering: 4 streams per LDW
            nc.tensor.matmul(uv0[:, 0, :], pr, er0, start=st, stop=sp)
            nc.tensor.matmul(uv1[:, 0, :], pr, er1, start=st, stop=sp)
            nc.tensor.matmul(wz0[:, 0, :], pr, ei0, start=st, stop=sp)
            nc.tensor.matmul(wz1[:, 0, :], pr, ei1, start=st, stop=sp)
            nc.tensor.matmul(uv0[:, 1, :], pi, ei0, start=st, stop=sp)
            nc.tensor.matmul(uv1[:, 1, :], pi, ei1, start=st, stop=sp)
            nc.tensor.matmul(wz0[:, 1, :], pi, er0, start=st, stop=sp)
            nc.tensor.matmul(wz1[:, 1, :], pi, er1, start=st, stop=sp)

        # U=uv[:,0], V=uv[:,1], W=wz[:,0], Z=wz[:,1].  ACT copies all four
        # to SBUF so the reversed right-half combines are SBUF-only and can
        # run on GPSIMD (which cannot read PSUM); left combines on DVE read
        # U/Z straight from PSUM (one-PSUM-operand rule).
        uvz0 = stgp.tile([128, 4, NTILE], f32, tag="vw")
        nc.scalar.copy(out=uvz0[:, 0, :], in_=uv0[:, 1, :])  # V
        nc.scalar.copy(out=uvz0[:, 1, :], in_=wz0[:, 0, :])  # W
        nc.scalar.copy(out=uvz0[:, 2, :], in_=uv0[:, 0, :])  # U
        nc.scalar.copy(out=uvz0[:, 3, :], in_=wz0[:, 1, :])  # Z
        stgL0 = stgp.tile([128, NTILE, 2], f32, tag="stg")
        nc.vector.tensor_sub(stgL0[:, :, 0], uv0[:, 0, :], uvz0[:, 0, :])
        nc.vector.tensor_add(stgL0[:, :, 1], wz0[:, 1, :], uvz0[:, 1, :])
        _dma_out(nc, out_t, stgL0, o, 0, skip_out)

        uvz1 = stgp.tile([128, 4, NTILE], f32, tag="vw")
        nc.scalar.copy(out=uvz1[:, 0, :], in_=uv1[:, 1, :])
        nc.scalar.copy(out=uvz1[:, 1, :], in_=wz1[:, 0, :])
        nc.scalar.copy(out=uvz1[:, 2, :], in_=uv1[:, 0, :])
        nc.scalar.copy(out=uvz1[:, 3, :], in_=wz1[:, 1, :])
        stgL1 = stgp.tile([128, NTILE, 2], f32, tag="stg")
        nc.vector.tensor_sub(stgL1[:, :, 0], uv1[:, 0, :], uvz1[:, 0, :])
        nc.vector.tensor_add(stgL1[:, :, 1], wz1[:, 1, :], uvz1[:, 1, :])
        _dma_out(nc, out_t, stgL1, o, 1, skip_out)

        # right tile 1: n in [2560, 3072) mirrors left tile 0
        stgR1 = stgp.tile([128, NTILE, 2], f32, tag="stg")
        nc.gpsimd.tensor_add(stgR1[:, 1:NTILE, 0],
                             _rev_ap(uvz0[:, 2, :], NTILE - 1, NTILE - 1),
                             _rev_ap(uvz0[:, 0, :], NTILE - 1, NTILE - 1))
        nc.gpsimd.tensor_sub(stgR1[:, 1:NTILE, 1],
                             _rev_ap(uvz0[:, 3, :], NTILE - 1, NTILE - 1),
                             _rev_ap(uvz0[:, 1, :], NTILE - 1, NTILE - 1))
        nc.gpsimd.tensor_add(stgR1[:, 0:1, 0], uvz1[:, 2, 0:1], uvz1[:, 0, 0:1])
        nc.gpsimd.tensor_sub(stgR1[:, 0:1, 1], uvz1[:, 3, 0:1], uvz1[:, 1, 0:1])
        _dma_out(nc, out_t, stgR1, o, 3, skip_out)

        # right tile 0: n in (2048, 2560) mirrors left tile 1; col 0 (the
        # self-mirrored n=2048 column) is written separately at body end
        stgR0 = stgp.tile([128, NTILE, 2], f32, tag="stg")
        nc.gpsimd.tensor_add(stgR0[:, 1:NTILE, 0],
                             _rev_ap(uvz1[:, 2, :], NTILE - 1, NTILE - 1),
                             _rev_ap(uvz1[:, 0, :], NTILE - 1, NTILE - 1))
        nc.gpsimd.tensor_sub(stgR0[:, 1:NTILE, 1],
                             _rev_ap(uvz1[:, 3, :], NTILE - 1, NTILE - 1),
                             _rev_ap(uvz1[:, 1, :], NTILE - 1, NTILE - 1))
        if not skip_out:
            for bl in range(BPC):
                nc.sync.dma_start(
                    out=out_t[bl, NV * o:NV * (o + 1),
                              2 * NTILE + 1:3 * NTILE, :],
                    in_=stgR0[NV * bl:NV * (bl + 1), 1:NTILE, :],
                )

    # ---- center column n=2048 (out col 1024): ctr = sum_k P * (-1)^p ----
    if not (skip_ctr or skip_mm):
        ctr = ps_m.tile([128, 2, NTILE], f32, tag="uv", name="ctr")
        for o in range(NO):
            nc.tensor.matmul(ctr[:, 0, o:o + 1], psum_o[o][:, 0, :],
                             epm_sb, start=True, stop=True)
            nc.tensor.matmul(ctr[:, 1, o:o + 1], psum_o[o][:, 1, :],
                             epm_sb, start=True, stop=True)
        ctr_sb = pfix.tile([128, 2, NO], f32, tag="ctrsb")
        nc.vector.tensor_copy(out=ctr_sb, in_=ctr[:, :, 0:NO])
        if not skip_out:
            for o in range(NO):
                for bl in range(BPC):
                    nc.sync.dma_start(
                        out=out_t[bl, NV * o:NV * (o + 1), 2 * NTILE, :],
                        in_=bass.AP(
                            ctr_sb.tensor,
                            ctr_sb.offset + NV * bl * ctr_sb.ap[0][0] + o,
                            [[ctr_sb.ap[0][0], NV], [NO, 2]],
                        ),
                    )


def _dma_out(nc, out_t, stg, o, nt, skip_out):
    if skip_out:
        return
    for bl in range(BPC):
        nc.sync.dma_start(
            out=out_t[bl, NV * o:NV * (o + 1), NTILE * nt:NTILE * (nt + 1), :],
            in_=stg[NV * bl:NV * (bl + 1), :, :],
        )


def _get_program(Psih, reps=1, variant="full"):
    key = f"prog{reps}_{variant}"
    if key not in _CACHE:
        if "consts" not in _CACHE:
            _CACHE["consts"] = _host_constants(np.asarray(Psih))
        w128_dev, tw_dev, c32blk, id128, e_dev, epm_dev, psiht, bands = _CACHE["consts"]
        nc = _build_program(w128_dev, tw_dev, c32blk, id128, e_dev, epm_dev,
                            bands, reps=reps, variant=variant)
        _CACHE[key] = (nc, psiht)
    return _CACHE[key]


def kernel(x, Psih=None, **_unused):
    x = np.ascontiguousarray(np.asarray(x), dtype=np.float32)
    if Psih is None:
        raise ValueError("Psih input required")
    nc, psiht = _get_program(Psih)
    in_maps = [
        {"x": np.ascontiguousarray(x[BPC * c:BPC * (c + 1)]), "psiht": psiht}
        for c in range(NC)
    ]
    res = run_bass_kernel_spmd(nc, in_maps, core_ids=list(range(NC)))
    out = np.concatenate([r["out"] for r in res.results], axis=0)
    return out.view(np.complex64)[..., 0]


def bench(x, Psih, iters=20, reps=1, variant="full"):
    """Run the kernel repeatedly on-device; returns (out_complex, times_ns)."""
    import time
    import jax
    from jax.sharding import Mesh, PartitionSpec
    from jax.experimental.shard_map import shard_map
    from concourse import bass2jax

    x = np.ascontiguousarray(np.asarray(x), dtype=np.float32)
    nc, psiht = _get_program(Psih, reps=reps, variant=variant)
    bass2jax.install_neuronx_cc_hook()

    part_name = nc.partition_id_tensor.name if nc.partition_id_tensor else None
    in_names, out_names, out_avals = [], [], []
    for alloc in nc.m.functions[0].allocations:
        if not isinstance(alloc, mybir.MemoryLocationSet):
            continue
        name = alloc.memorylocations[0].name
        if alloc.kind == "ExternalInput":
            if name != part_name:
                in_names.append(name)
        elif alloc.kind == "ExternalOutput":
            out_names.append(name)
            out_avals.append(
                jax.core.ShapedArray(
                    tuple(alloc.tensor_shape), mybir.dt.np(alloc.dtype)
                )
            )
    n_params = len(in_names)
    all_names = in_names + out_names
    if part_name is not None:
        all_names = all_names + [part_name]

    def _body(*args):
        operands = list(args)
        if part_name is not None:
            operands.append(bass2jax.partition_id_tensor())
        outs = bass2jax._bass_exec_p.bind(
            *operands,
            out_avals=tuple(out_avals),
            in_names=tuple(all_names),
            out_names=tuple(out_names),
            lowering_input_output_aliases=(),
            sim_require_finite=True,
            sim_require_nnan=True,
            nc=nc,
        )
        return tuple(outs)

    devices = jax.devices()[:NC]
    mesh = Mesh(np.asarray(devices), ("core",))
    nin = n_params + len(out_names)
    fn = jax.jit(
        shard_map(
            _body,
            mesh=mesh,
            in_specs=(PartitionSpec("core"),) * nin,
            out_specs=(PartitionSpec("core"),) * len(out_names),
            check_rep=False,
        ),
        keep_unused=True,
    )
    in_map = {"x": x, "psiht": np.concatenate([psiht] * NC, axis=0)}
    concat_in = [in_map[n] for n in in_names]
    concat_zeros = [
        np.zeros((NC * a.shape[0], *a.shape[1:]), a.dtype) for a in out_avals
    ]
    sharding = jax.sharding.NamedSharding(mesh, PartitionSpec("core"))
    args = [jax.device_put(a, sharding) for a in concat_in + concat_zeros]
    out_arrs = jax.block_until_ready(fn(*args))  # compile + first run
    times = []
    for _ in range(iters):
        t0 = time.perf_counter()
        out_arrs = jax.block_until_ready(fn(*args))
        times.append((time.perf_counter() - t0) * 1e9)
    out = np.asarray(out_arrs[0]).reshape(NC, BPC, NA, L, 2).reshape(B, NA, L, 2)
    return out.view(np.complex64)[..., 0], times


# revision 11
# speedup vs baseline: 1.7075x; 1.2542x over previous
"""CWT (GMW filterbank) Trainium2 kernel, v2.

Computes Wx = ifft(Psih * fft(reflect_pad(x)))[..., N1:N1+L] for
x (32, 2048) f32, Psih (256, 4096) f32 -> out (32, 256, 2048) complex64.

Strategy (8 NeuronCores, data-parallel over batch, 4 rows/core):
  - Forward DFT via Cooley-Tukey 4096 = 128 x 32 (DIT): reflect pad is
    materialized by 3 region DMAs + a DVE inner-reversal (the n1-row
    permutation this induces is baked into the stage-1 DFT-128 weights).
    Stage 1 = one [128x128] matmul pair (fp32), twiddle on DVE, one PE
    transpose pair, stage 2 = DFT-32 as a block-diagonal-over-batch
    [128x64] matmul quad (fp32).  ~4 us PE vs ~55 us for the dense DFT,
    and no 16 MB/iteration DFT-matrix HBM traffic.
  - P = Psih (.) xh in fp16 (pre-scaled by 1/4096 via the stage-2
    constants so E can be stored as raw +-1-range phases; avoids fp16
    subnormals).  re on DVE, im on GPSIMD.
  - Banded inverse DFT in fp16 (same PE speed as bf16, 8x the mantissa):
    per-octave k-tile bands at threshold 5e-3 (29 (octave,ktile) pairs).
    Mirror symmetry E[k, 4096-n] = conj(E[k, n]) halves the matmuls:
    U/V/W/Z accumulate over n in [1024, 2048) only; right-half outputs
    assembled with reversed APs.  E tiles persist in SBUF across the
    rep loop (loaded once in the prologue).
  - The 8 concurrent PSUM accumulation groups (U,V,W,Z x 2 n-tiles) map
    1:1 onto the 8 PSUM banks.  start=True clears has_written for the
    whole bank, so interleaved groups MUST be bank-disjoint.
  - Center column n=2048 (self-mirror): per-octave (-1)^p projections of
    P emitted at body end (58 LDW-bound matmuls overlapping the next
    iteration's P-gen via the For_i pipeline), written by 32 tiny
    PSUM->DRAM DMAs.
  - Mirror assembly: V/W copies on ACT (scalar), left combines on DVE,
    right (reversed) combines on GPSIMD; outputs DMA out as
    (b, a, n, 2) f32 in 4x 32-partition chunks to spread HWDGE queues.

Build notes inherited from v1 (hard-won):
  - Use bacc.Bacc() + nc.compile(): plain bass.Bass() fails walrus
    codegen with "Too many sync wait commands".
  - DVE tensor_tensor may read only ONE operand from PSUM.
  - DVE/ACT/GPSIMD are partition-locked; partition permutations need
    DMA or PE transposes (or host-side constant permutation).
  - A single big out-DMA serializes on one HWDGE queue.
"""

import numpy as np

import concourse.bass as bass
import concourse.bacc as bacc
import concourse.mybir as mybir
import concourse.tile as tile
from concourse.bass_utils import run_bass_kernel_spmd

B = 32          # batch
L = 2048        # signal length
UP = 4096       # padded length
N1 = 1024       # left pad (slice offset)
NA = 256        # scales
NV = 32         # voices/octave
NO = 8          # octaves
KF = 2048       # used frequency bins
NC = 8          # cores
BPC = B // NC   # batch rows per core (4)
KT = KF // 128  # k tiles (16)
NTILE = 512     # output columns per matmul
BAND_THRESH = 5e-3

_CACHE = {}


def _host_constants(Psih):
    """FFT stage constants, E filter-bank tiles (fp16), band table."""
    # stage-1 DFT-128 weights with the n1 partition permutation induced by
    # the staged reflect-pad load (regions 1/3 land with n1 reversed)
    n1_of_p = np.arange(128)
    n1_of_p[0:32] = 31 - np.arange(32)
    n1_of_p[96:128] = 223 - np.arange(96, 128)
    th = 2 * np.pi * n1_of_p[:, None] * np.arange(128)[None, :] / 128
    w128_dev = np.stack([np.cos(th), -np.sin(th)], 1).astype(np.float32)

    # twiddle T[k1, n2] = exp(-2i pi k1 n2 / 4096)
    tt = 2 * np.pi * np.arange(128)[:, None] * np.arange(32)[None, :] / UP
    tw_dev = np.stack([np.cos(tt), -np.sin(tt)], 1).astype(np.float32)

    # stage-2 DFT-32 block-diagonal over batch, cols k2 < 16 (k < 2048),
    # scaled by 1/UP so xh (hence P) carries the ifft normalization
    t2 = 2 * np.pi * np.arange(32)[:, None] * np.arange(16)[None, :] / 32
    C32 = np.cos(t2) / UP
    S32 = np.sin(t2) / UP
    c32blk = np.zeros((128, 3, 64), np.float32)
    for b in range(BPC):
        c32blk[32 * b:32 * b + 32, 0, 16 * b:16 * b + 16] = C32
        c32blk[32 * b:32 * b + 32, 1, 16 * b:16 * b + 16] = S32
        c32blk[32 * b:32 * b + 32, 2, 16 * b:16 * b + 16] = -S32

    id128 = np.eye(128, dtype=np.float32)

    # E tiles: raw phases (x UP vs the true inverse-DFT matrix; the 1/UP
    # lives in c32blk): E'[k, n] = exp(2i pi k n / UP), n in [N1, N1+L)
    kk = np.arange(KF)[:, None]
    nn = np.arange(N1, N1 + L // 2)[None, :]
    ph = 2 * np.pi * kk * nn / UP
    Er = np.cos(ph).reshape(KT, 128, 2, NTILE)
    Ei = np.sin(ph).reshape(KT, 128, 2, NTILE)
    e_dev = np.empty((2, KT, 128, 2, NTILE), dtype=np.float16)
    e_dev[:, :, :, 0, :] = Er.transpose(2, 0, 1, 3).astype(np.float16)
    e_dev[:, :, :, 1, :] = Ei.transpose(2, 0, 1, 3).astype(np.float16)

    # center column: E'[k, 2048] = (-1)^k, independent of kt
    epm_dev = ((-1.0) ** (np.arange(128) % 2)).astype(np.float16)[:, None]

    # PsihT device layout: (k_in 128, kt, a 256) f32
    psiht = np.ascontiguousarray(
        Psih[:, :KF].T.reshape(KT, 128, NA).transpose(1, 0, 2)
    ).astype(np.float16)

    bands = []
    for o in range(NO):
        sub = Psih[NV * o:NV * (o + 1), :KF]
        ks = np.nonzero((sub > BAND_THRESH * 2.0).any(axis=0))[0]
        bands.append((int(ks.min()) // 128, int(ks.max()) // 128 + 1))

    return w128_dev, tw_dev, c32blk, id128, e_dev, epm_dev, psiht, bands


def _rev_ap(ap2d, last_col, count):
    """Columns [last_col, last_col-1, ...] of a [128, C] AP."""
    return bass.AP(
        ap2d.tensor,
        ap2d.offset + last_col * ap2d.ap[-1][0],
        [list(ap2d.ap[0]), [-ap2d.ap[-1][0], count]],
    )


def _rev_inner(ap3, n):
    """Reverse the innermost dim (size n) of a 3-dim AP."""
    return bass.AP(
        ap3.tensor, ap3.offset + (n - 1) * ap3.ap[-1][0],
        [list(ap3.ap[0]), list(ap3.ap[1]), [-ap3.ap[-1][0], n]],
    )


def _build_program(w128_dev, tw_dev, c32blk, id128, e_dev, epm_dev, bands,
                   reps=1, variant="full"):
    f32 = mybir.dt.float32
    fp16 = mybir.dt.float16

    nc = bacc.Bacc()
    x_in = nc.dram_tensor("x", [BPC, L], f32, kind="ExternalInput")
    psih_in = nc.dram_tensor("psiht", [128, KT, NA], fp16, kind="ExternalInput")
    out_t = nc.dram_tensor("out", [NO, 4, BPC, NV, NTILE, 2], f32, kind="ExternalOutput")
    ctr_t = nc.dram_tensor("ctrout", [128, 2, NO], f32, kind="ExternalOutput")

    w128_c = nc.inline_tensor(w128_dev, name="w128c")
    tw_c = nc.inline_tensor(tw_dev, name="twc")
    c32_c = nc.inline_tensor(c32blk, name="c32c")
    id_c = nc.inline_tensor(id128, name="idc")
    e_c = nc.inline_tensor(e_dev, name="econst")
    epm_c = nc.inline_tensor(epm_dev, name="epmc")

    with tile.TileContext(nc) as tc:
        with (
            tc.tile_pool(name="persist", bufs=1) as persist,
            tc.tile_pool(name="pfix", bufs=1) as pfix,
            tc.tile_pool(name="stg", bufs=3) as stgp,
            tc.tile_pool(name="ps_m", bufs=2, space="PSUM") as ps_m,
        ):
            # ---- prologue: inputs + persistent constants ----
            xpt = persist.tile([128, BPC, 32], f32, tag="xp")
            s13 = persist.tile([128, BPC, 32], f32, tag="s13")
            for b in range(BPC):
                nc.sync.dma_start(
                    out=xpt[32:96, b],
                    in_=bass.AP(x_in, L * b, [[32, 64], [1, 32]]),
                )
                nc.sync.dma_start(
                    out=s13[0:32, b],
                    in_=bass.AP(x_in, L * b + 1, [[32, 32], [1, 32]]),
                )
                nc.sync.dma_start(
                    out=s13[96:128, b],
                    in_=bass.AP(x_in, L * b + 1023, [[32, 32], [1, 32]]),
                )
            nc.vector.tensor_copy(out=xpt[0:32], in_=_rev_inner(s13[0:32], 32))
            nc.vector.tensor_copy(out=xpt[96:128], in_=_rev_inner(s13[96:128], 32))

            psih_sb = persist.tile([128, KT, NA], fp16, tag="psih")
            nc.sync.dma_start(out=psih_sb, in_=psih_in[:])
            # funnel: absorb the psih DMA wait into DVE's clock
            scratch = persist.tile([1, 4], f32, tag="scratch")
            nc.vector.tensor_copy(out=scratch[0:1, 0:1], in_=psih_sb[0:1, 0, 0:1])

            w128_sb = persist.tile([128, 2, 128], f32, tag="w128")
            nc.sync.dma_start(out=w128_sb, in_=w128_c[:])
            tw_sb = persist.tile([128, 2, 32], f32, tag="tw")
            nc.sync.dma_start(out=tw_sb, in_=tw_c[:])
            c32_sb = persist.tile([128, 3, 64], f32, tag="c32")
            nc.sync.dma_start(out=c32_sb, in_=c32_c[:])
            id_sb = persist.tile([128, 128], f32, tag="id")
            nc.sync.dma_start(out=id_sb, in_=id_c[:])
            epm_sb = persist.tile([128, 1], fp16, tag="epm")
            nc.sync.dma_start(out=epm_sb, in_=epm_c[:])

            etiles = {}
            for lnt in range(2):
                for kt in range(KT):
                    et = persist.tile([128, 2, NTILE], fp16, tag=f"e{lnt}_{kt}")
                    nc.sync.dma_start(out=et, in_=e_c[lnt, kt])
                    etiles[(lnt, kt)] = et

            # dummy transpose absorbs the identity-DMA wait on PE
            dmy = ps_m.tile([128, 2, NTILE], f32, tag="uv", name="dmy")
            nc.tensor.transpose(dmy[:, 0, 0:128], id_sb, id_sb)

            def body():
                _emit_body(
                    nc, bands, out_t, ctr_t, persist, pfix, stgp, ps_m,
                    xpt, psih_sb, w128_sb, tw_sb, c32_sb, id_sb, epm_sb,
                    etiles, f32, fp16, variant,
                )

            if reps == 1:
                body()
            else:
                with tc.For_i(0, reps, 1):
                    body()
    nc.compile()
    return nc


def _emit_body(nc, bands, out_t, ctr_t, persist, pfix, stgp, ps_m,
               xpt, psih_sb, w128_sb, tw_sb, c32_sb, id_sb, epm_sb,
               etiles, f32, fp16, variant):
    skip_fwd = "nofwd" in variant
    skip_out = "noout" in variant
    skip_mm = "nomm" in variant
    skip_ctr = "noctr" in variant

    # ---- forward FFT (one ps_m "uv" slot hosts all fwd psum) ----
    xh_all = persist.tile([128, 2, BPC, KT], fp16, tag="xh")
    if not skip_fwd:
        fwd = ps_m.tile([128, 2, NTILE], f32, tag="uv", name="fwd")
        bre = fwd[:, 0, 0:128]
        bim = fwd[:, 1, 0:128]
        xp_flat = xpt.rearrange("p b n -> p (b n)")
        nc.tensor.matmul(bre, w128_sb[:, 0, :], xp_flat, start=True, stop=True)
        nc.tensor.matmul(bim, w128_sb[:, 1, :], xp_flat, start=True, stop=True)

        dd = pfix.tile([128, 2, BPC, 32], f32, tag="dd")
        t1 = pfix.tile([128, BPC, 32], f32, tag="t1")
        t2 = pfix.tile([128, BPC, 32], f32, tag="t2")
        brr = bre.rearrange("p (b n) -> p b n", b=BPC)
        bir = bim.rearrange("p (b n) -> p b n", b=BPC)
        tre = tw_sb[:, 0, None, :].to_broadcast((128, BPC, 32))
        tim = tw_sb[:, 1, None, :].to_broadcast((128, BPC, 32))
        nc.vector.tensor_tensor(t1, brr, tre, mybir.AluOpType.mult)
        nc.vector.tensor_tensor(t2, bir, tim, mybir.AluOpType.mult)
        nc.vector.tensor_sub(dd[:, 0], t1, t2)
        nc.vector.tensor_tensor(t1, brr, tim, mybir.AluOpType.mult)
        nc.vector.tensor_tensor(t2, bir, tre, mybir.AluOpType.mult)
        nc.vector.tensor_add(dd[:, 1], t1, t2)

        dtp_re = fwd[:, 0, 128:256]
        dtp_im = fwd[:, 1, 128:256]
        nc.tensor.transpose(dtp_re, dd[:, 0].rearrange("p b n -> p (b n)"), id_sb)
        nc.tensor.transpose(dtp_im, dd[:, 1].rearrange("p b n -> p (b n)"), id_sb)
        dts = pfix.tile([128, 2, 128], f32, tag="dts")
        nc.scalar.copy(out=dts[:, 0, :], in_=dtp_re)
        nc.scalar.copy(out=dts[:, 1, :], in_=dtp_im)

        xre = fwd[:, 0, 256:320]
        xim = fwd[:, 1, 256:320]
        nc.tensor.matmul(xre, dts[:, 0, :], c32_sb[:, 0, :], start=True, stop=False)
        nc.tensor.matmul(xim, dts[:, 0, :], c32_sb[:, 2, :], start=True, stop=False)
        nc.tensor.matmul(xre, dts[:, 1, :], c32_sb[:, 1, :], start=False, stop=True)
        nc.tensor.matmul(xim, dts[:, 1, :], c32_sb[:, 0, :], start=False, stop=True)
        nc.vector.tensor_copy(
            out=xh_all.rearrange("p c b k -> p c (b k)"),
            in_=fwd[:, :, 256:320],
        )

    # ---- P generation, high kt first (octave 0's band is ready first).
    # All on DVE in fp16 (GPSIMD has multi-us per-op overhead).  Band sums
    # for the center column accumulate incrementally so the ctr matmuls at
    # body end never touch the P tiles (keeps the For_i pipeline free).
    P_re, P_im = {}, {}
    psum_o = {} if not skip_mm else None
    for kt in reversed(range(KT)):
        octs = [o for o in range(NO) if bands[o][0] <= kt < bands[o][1]]
        olo, ohi = min(octs), max(octs) + 1
        nocts = ohi - olo
        psih_ap = (
            psih_sb[:, kt, :]
            .rearrange("p (o a) -> p o a", o=NO)[:, olo:ohi, None, :]
            .to_broadcast((128, nocts, BPC, NV))
        )
        for comp, dst in ((0, P_re), (1, P_im)):
            pt = pfix.tile([128, NO * BPC * NV], fp16, tag=f"P{comp}_{kt}")
            out_ap = pt.rearrange("p (o b a) -> p o b a", o=NO, b=BPC)[:, olo:ohi]
            if skip_fwd:
                nc.vector.tensor_copy(out=out_ap, in_=psih_ap)
            else:
                xh_ap = (
                    xh_all[:, comp, :, kt][:, None, :, None]
                    .to_broadcast((128, nocts, BPC, NV))
                )
                nc.vector.tensor_tensor(
                    out_ap, psih_ap, xh_ap, mybir.AluOpType.mult,
                )
            dst[kt] = pt
        if psum_o is not None:
            for o in range(NO):
                klo, khi = bands[o]
                if not (klo <= kt < khi):
                    continue
                osl = slice(o * 128, (o + 1) * 128)
                if o not in psum_o:
                    psum_o[o] = pfix.tile([128, 2, 128], fp16,
                                          tag=f"psum{o}", bufs=2,
                                          name=f"psum{o}")
                ps_t = psum_o[o]
                for comp, src_t in ((0, P_re[kt]), (1, P_im[kt])):
                    if kt == khi - 1:
                        nc.vector.tensor_copy(out=ps_t[:, comp, :],
                                              in_=src_t[:, osl])
                    else:
                        nc.vector.tensor_add(ps_t[:, comp, :],
                                             ps_t[:, comp, :], src_t[:, osl])

    # ---- banded mirror inverse DFT ----
    for o in range(NO):
        if skip_mm:
            continue
        klo, khi = bands[o]
        kts = list(range(klo, khi))
        osl = slice(o * 128, (o + 1) * 128)
        uv0 = ps_m.tile([128, 2, NTILE], f32, tag="uv", name="uv0")
        wz0 = ps_m.tile([128, 2, NTILE], f32, tag="wz", name="wz0")
        uv1 = ps_m.tile([128, 2, NTILE], f32, tag="uv", name="uv1")
        wz1 = ps_m.tile([128, 2, NTILE], f32, tag="wz", name="wz1")
        for j, kt in enumerate(kts):
            st, sp = (j == 0), (j == len(kts) - 1)
            pr = P_re[kt][:, osl]
            pi = P_im[kt][:, osl]
            er0 = etiles[(0, kt)][:, 0, :]
            ei0 = etiles[(0, kt)][:, 1, :]
            er1 = etiles[(1, kt)][:, 0, :]
            ei1 = etiles[(1, kt)][:, 1, :]
            # stationary-reuse ordering: 4 streams per LDW
            nc.tensor.matmul(uv0[:, 0, :], pr, er0, start=st, stop=sp)
            nc.tensor.matmul(uv1[:, 0, :], pr, er1, start=st, stop=sp)
            nc.tensor.matmul(wz0[:, 0, :], pr, ei0, start=st, stop=sp)
            nc.tensor.matmul(wz1[:, 0, :], pr, ei1, start=st, stop=sp)
            nc.tensor.matmul(uv0[:, 1, :], pi, ei0, start=st, stop=sp)
            nc.tensor.matmul(uv1[:, 1, :], pi, ei1, start=st, stop=sp)
            nc.tensor.matmul(wz0[:, 1, :], pi, er0, start=st, stop=sp)
            nc.tensor.matmul(wz1[:, 1, :], pi, er1, start=st, stop=sp)

        # U=uv[:,0], V=uv[:,1], W=wz[:,0], Z=wz[:,1].  ACT copies all four
        # to SBUF so the reversed right-half combines are SBUF-only and can
        # run on GPSIMD (which cannot read PSUM); left combines on DVE read
        # U/Z straight from PSUM (one-PSUM-operand rule).
        uvz0 = stgp.tile([128, 2, NTILE], f32, tag="vw0")
        nc.scalar.copy(out=uvz0[:, 0, :], in_=uv0[:, 1, :])  # V
        nc.scalar.copy(out=uvz0[:, 1, :], in_=wz0[:, 0, :])  # W
        stgL0 = stgp.tile([128, NTILE, 2], f32, tag="stg")
        nc.vector.tensor_sub(stgL0[:, :, 0], uv0[:, 0, :], uvz0[:, 0, :])
        nc.vector.tensor_add(stgL0[:, :, 1], wz0[:, 1, :], uvz0[:, 1, :])
        _dma_out(nc, out_t, stgL0, o, 0, skip_out)

        uvz1 = stgp.tile([128, 4, NTILE], f32, tag="vw")
        nc.scalar.copy(out=uvz1[:, 0, :], in_=uv1[:, 1, :])
        nc.scalar.copy(out=uvz1[:, 1, :], in_=wz1[:, 0, :])
        nc.scalar.copy(out=uvz1[:, 2, :], in_=uv1[:, 0, :])
        nc.scalar.copy(out=uvz1[:, 3, :], in_=wz1[:, 1, :])
        stgL1 = stgp.tile([128, NTILE, 2], f32, tag="stg")
        nc.vector.tensor_sub(stgL1[:, :, 0], uv1[:, 0, :], uvz1[:, 0, :])
        nc.vector.tensor_add(stgL1[:, :, 1], wz1[:, 1, :], uvz1[:, 1, :])
        _dma_out(nc, out_t, stgL1, o, 1, skip_out)

        # right tile 1: n in [2560, 3072) mirrors left tile 0 (DVE reads
        # U/Z straight from PSUM with reversed APs; V/W from SBUF)
        stgR1 = stgp.tile([128, NTILE, 2], f32, tag="stg")
        nc.vector.tensor_add(stgR1[:, 1:NTILE, 0],
                             _rev_ap(uv0[:, 0, :], NTILE - 1, NTILE - 1),
                             _rev_ap(uvz0[:, 0, :], NTILE - 1, NTILE - 1))
        nc.vector.tensor_sub(stgR1[:, 1:NTILE, 1],
                             _rev_ap(wz0[:, 1, :], NTILE - 1, NTILE - 1),
                             _rev_ap(uvz0[:, 1, :], NTILE - 1, NTILE - 1))
        nc.vector.tensor_add(stgR1[:, 0:1, 0], uvz1[:, 2, 0:1], uvz1[:, 0, 0:1])
        nc.vector.tensor_sub(stgR1[:, 0:1, 1], uvz1[:, 3, 0:1], uvz1[:, 1, 0:1])
        _dma_out(nc, out_t, stgR1, o, 3, skip_out)

        # right tile 0: n in (2048, 2560) mirrors left tile 1; col 0 (the
        # self-mirrored n=2048 column) is written separately at body end
        stgR0 = stgp.tile([128, NTILE, 2], f32, tag="stg")
        nc.gpsimd.tensor_add(stgR0[:, 1:NTILE, 0],
                             _rev_ap(uvz1[:, 2, :], NTILE - 1, NTILE - 1),
                             _rev_ap(uvz1[:, 0, :], NTILE - 1, NTILE - 1))
        nc.gpsimd.tensor_sub(stgR0[:, 1:NTILE, 1],
                             _rev_ap(uvz1[:, 3, :], NTILE - 1, NTILE - 1),
                             _rev_ap(uvz1[:, 1, :], NTILE - 1, NTILE - 1))
        if not skip_out:
            nc.sync.dma_start(out=out_t[o, 2, :, :, 1:NTILE, :],
                              in_=stgR0[:, 1:NTILE, :])

    # ---- center column n=2048 (out col 1024): ctr = sum_k P * (-1)^p ----
    if not (skip_ctr or skip_mm):
        ctr = ps_m.tile([128, 2, NTILE], f32, tag="uv", name="ctr")
        for o in range(NO):
            nc.tensor.matmul(ctr[:, 0, o:o + 1], psum_o[o][:, 0, :],
                             epm_sb, start=True, stop=True)
            nc.tensor.matmul(ctr[:, 1, o:o + 1], psum_o[o][:, 1, :],
                             epm_sb, start=True, stop=True)
        ctr_sb = pfix.tile([128, 2, NO], f32, tag="ctrsb")
        nc.vector.tensor_copy(out=ctr_sb, in_=ctr[:, :, 0:NO])
        if not skip_out:
            nc.sync.dma_start(out=ctr_t[:], in_=ctr_sb)


def _dma_out(nc, out_t, stg, o, nt, skip_out):
    if skip_out:
        return
    nc.sync.dma_start(out=out_t[o, nt], in_=stg)


def _get_program(Psih, reps=1, variant="full"):
    key = f"prog{reps}_{variant}"
    if key not in _CACHE:
        if "consts" not in _CACHE:
            _CACHE["consts"] = _host_constants(np.asarray(Psih))
        w128_dev, tw_dev, c32blk, id128, e_dev, epm_dev, psiht, bands = _CACHE["consts"]
        nc = _build_program(w128_dev, tw_dev, c32blk, id128, e_dev, epm_dev,
                            bands, reps=reps, variant=variant)
        _CACHE[key] = (nc, psiht)
    return _CACHE[key]


def kernel(x, Psih=None, **_unused):
    x = np.ascontiguousarray(np.asarray(x), dtype=np.float32)
    if Psih is None:
        raise ValueError("Psih input required")
    nc, psiht = _get_program(Psih)
    in_maps = [
        {"x": np.ascontiguousarray(x[BPC * c:BPC * (c + 1)]), "psiht": psiht}
        for c in range(NC)
    ]
    res = run_bass_kernel_spmd(nc, in_maps, core_ids=list(range(NC)))
    # device layout [NO, nt, BPC, NV, NTILE, 2] -> [BPC, NA, L, 2];
    # n=2048 center column ships separately as ctrout [128 (b,a), 2, NO]
    outs = []
    for r in res.results:
        o = r["out"].transpose(2, 0, 3, 1, 4, 5).reshape(BPC, NA, L, 2)
        c = r["ctrout"].reshape(BPC, NV, 2, NO).transpose(0, 3, 1, 2)
        o[:, :, 1024, :] = c.reshape(BPC, NA, 2)
        outs.append(o)
    out = np.ascontiguousarray(np.concatenate(outs, axis=0))
    return out.view(np.complex64)[..., 0]


def bench(x, Psih, iters=20, reps=1, variant="full"):
    """Run the kernel repeatedly on-device; returns (out_complex, times_ns)."""
    import time
    import jax
    from jax.sharding import Mesh, PartitionSpec
    from jax.experimental.shard_map import shard_map
    from concourse import bass2jax

    x = np.ascontiguousarray(np.asarray(x), dtype=np.float32)
    nc, psiht = _get_program(Psih, reps=reps, variant=variant)
    bass2jax.install_neuronx_cc_hook()

    part_name = nc.partition_id_tensor.name if nc.partition_id_tensor else None
    in_names, out_names, out_avals = [], [], []
    for alloc in nc.m.functions[0].allocations:
        if not isinstance(alloc, mybir.MemoryLocationSet):
            continue
        name = alloc.memorylocations[0].name
        if alloc.kind == "ExternalInput":
            if name != part_name:
                in_names.append(name)
        elif alloc.kind == "ExternalOutput":
            out_names.append(name)
            out_avals.append(
                jax.core.ShapedArray(
                    tuple(alloc.tensor_shape), mybir.dt.np(alloc.dtype)
                )
            )
    n_params = len(in_names)
    all_names = in_names + out_names
    if part_name is not None:
        all_names = all_names + [part_name]

    def _body(*args):
        operands = list(args)
        if part_name is not None:
            operands.append(bass2jax.partition_id_tensor())
        outs = bass2jax._bass_exec_p.bind(
            *operands,
            out_avals=tuple(out_avals),
            in_names=tuple(all_names),
            out_names=tuple(out_names),
            lowering_input_output_aliases=(),
            sim_require_finite=True,
            sim_require_nnan=True,
            nc=nc,
        )
        return tuple(outs)

    devices = jax.devices()[:NC]
    mesh = Mesh(np.asarray(devices), ("core",))
    nin = n_params + len(out_names)
    fn = jax.jit(
        shard_map(
            _body,
            mesh=mesh,
            in_specs=(PartitionSpec("core"),) * nin,
            out_specs=(PartitionSpec("core"),) * len(out_names),
            check_rep=False,
        ),
        keep_unused=True,
    )
    in_map = {"x": x, "psiht": np.concatenate([psiht] * NC, axis=0)}
    concat_in = [in_map[n] for n in in_names]
    concat_zeros = [
        np.zeros((NC * a.shape[0], *a.shape[1:]), a.dtype) for a in out_avals
    ]
    sharding = jax.sharding.NamedSharding(mesh, PartitionSpec("core"))
    args = [jax.device_put(a, sharding) for a in concat_in + concat_zeros]
    out_arrs = jax.block_until_ready(fn(*args))  # compile + first run
    times = []
    for _ in range(iters):
        t0 = time.perf_counter()
        out_arrs = jax.block_until_ready(fn(*args))
        times.append((time.perf_counter() - t0) * 1e9)
    oidx = out_names.index("out")
    cidx = out_names.index("ctrout")
    out = np.asarray(out_arrs[oidx]).reshape(NC, NO, 4, BPC, NV, NTILE, 2)
    out = np.ascontiguousarray(out.transpose(0, 3, 1, 4, 2, 5, 6)).reshape(B, NA, L, 2)
    ctr = np.asarray(out_arrs[cidx]).reshape(NC, BPC, NV, 2, NO).transpose(0, 1, 4, 2, 3)
    out[:, :, 1024, :] = ctr.reshape(B, NA, 2)
    return out.view(np.complex64)[..., 0], times
